# revision 21
# baseline (speedup 1.0000x reference)
"""Trainium2 Bass kernel for GroupedKAAttention (fp16 datapath, v4).

Math per batch row b (B=4096 total, 512 per core, data-parallel over 8
NeuronCores, weights replicated):
  xg[b,g,:]  = x[b, g*64:(g+1)*64]                      (G=64 groups, D=64)
  h[b,g,:]   = silu(xg[b,g,:] @ W1[g] + b1[g])          (H=512)
  f[b,g,:]   = h[b,g,:] @ W2[g] + b2[g]                 (P=64 patches)
  h2[b,p,:]  = silu(f[b,:,p] @ Wg1 + bg1)               (contract groups)
  o[b,p,:]   = h2[b,p,:] @ Wg2 + bg2                    (E=16 heads)
  attn[b]    = sum_{p,e} o_q * o_k ;  out = softmax(attn over b)

v4: the scalar (Act) engine is the roofline (~67M silu evals/core, 98%
busy in v3).  A slice of the silu work moves to the idle DVE via two
runtime-registered custom DVE ops, using the identity
  silu(z) = z/2 + g(z),   g(z) = (z/2)tanh(z/2)  (EVEN in z)
g is approximated by a (2,2) rational in y=z^2 (weighted rms ~1e-5):
  g ~= n1*y*(y+c0) / (y^2 + d1*y + d0)
For an offloaded hidden-chunk (128 of 512 hidden units):
  1. y  = z*z                    stock tensor_tensor from PSUM  (658ns)
  2. r  ~= 1/D(y)                custom op: monic quadratic, bitwise-NOT
                                 reciprocal seed + 1 Newton step (8 ALU
                                 stages, 593ns)
  3. g  = n1*(y+c0)*y*r          custom op (4 stages, 593ns)
The missing linear z/2 rides the next GEMM exactly: the stationary W1/Wg1
tiles carry 64/32 extra columns holding W1c@W2c/2 (resp Wg1c@Wg2c/2) so
one extra accumulating matmul per offloaded group/patch adds z/2 @ W2.
OFF_G/OFF_P control how many groups/patches offload chunk 3 (balance Act
vs DVE).  End-to-end approx error at full offload of chunk3 everywhere
measured 5.4e-3 (gate 2e-2); at OFF_G=OFF_P=46 it is ~2-3e-3.

Other v4 changes: the u-tile ones row comes from a DMA (not a DVE copy);
the k-stream bias-add + q*k product is one fused custom DVE op.

`reps` unrolls the computation R times inside one NEFF (weights stay
SBUF-resident, f bounce double-buffered) for steady-state throughput
benchmarking; the correctness path uses reps=1.
"""

import numpy as np

B = 4096
TOTAL_DIM = 4096
G = 64            # groups
D = 64            # group size
H = 512           # hidden
P = 64            # patches
E = 16            # heads
NCORES = 8
BC = B // NCORES  # 512 batch rows per core
NPAIR = P // 2    # 32 patch pairs (global stage)

# how many of the 64 groups (grouped stage) / 64 patches (global stage)
# route hidden-chunk 3 through the DVE instead of the Act engine
OFF_G = 46
OFF_P = 46
# KA_ADDMUL with partition-shifted operands produced NaN on HW; the stock
# add+mul pair costs ~21us more DVE but is correct.
USE_KAM = False

# rational-fit constants per stage: (n1, c0, d1, d0, seed_const)
FIT1 = (11.37246959, 44.892305, 215.49627357, 2042.31513025, -0.235292)
FIT2 = (11.56992132, 44.93750663, 219.51844636, 2079.73021591, -0.235293)


def _offsel(i, n_off, n_tot=64):
    return (i * n_off) % n_tot < n_off


def _register_ops():
    """Register the custom DVE ops (idempotent across rebuilds)."""
    from concourse import dve_ops
    from concourse.dve_spec import (
        Spec, Src0, Src1, C0, C1, C2, Bin, AluOp, Zero, One, lower, _has_src1)
    from concourse.dve_uop import DveOpSpec

    def mk(name, spec):
        for o in dve_ops.OPS:
            if o.name == name:
                return o
        row = dve_ops._CUSTOM_DVE_ROW_BASE + len(dve_ops.OPS)
        dve_ops._SUB_OPCODE_FOR_NAME[name] = row
        shas = {}
        for ver in ("v3", "v4"):
            s = DveOpSpec(name=name, opcode=row, uops=lower(spec, ver=ver),
                          rd1_en=_has_src1(spec))
            shas[ver] = s.sha(ver)
        op = dve_ops.DveOp(name, spec, subdim=False, uops_sha=shas)
        dve_ops.OPS.append(op)
        dve_ops.CUSTOM_DVE_SPECS[name] = spec
        return op

    def _np_not(a):
        return (~np.asarray(a, np.float32).view(np.int32)).view(np.float32)

    from concourse.dve_spec import sq

    # D = (z^2 + C0)*z^2 + C1  — single PSUM read (PSUM has one DVE port)
    yq = sq(Src0)
    deval_spec = Spec(
        body=(yq + C0) * yq + C1,
        reference=lambda in0, s0, s1, imm2: (
            lambda y: (y + s0) * y + s1)(np.square(np.asarray(in0, np.float32))),
    )
    recip_op = mk("KA_DEVAL", deval_spec)

    # r ~= 1/D: bitwise-NOT seed * C0, then two Newton steps. 8 ALU stages.
    seed = Bin(AluOp.BITWISE_NOT, Src0, Src0) * C0
    r1 = seed * ((One + One) - Src0 * seed)
    recip2_spec = Spec(
        body=r1 * ((One + One) - Src0 * r1),
        reference=lambda in0, s0, s1, imm2: (
            lambda Dv: (lambda s: (lambda a: a * (2.0 - Dv * a))(
                s * (2.0 - Dv * s)))(_np_not(Dv) * s0)
        )(np.asarray(in0, np.float32)),
    )
    recip2_op = mk("KA_RECIP2", recip2_spec)

    # g = ((z^2+C0)*z^2) * r * C1    (5 stages; z from PSUM, r elementwise)
    yg = sq(Src0)
    geval_spec = Spec(
        body=((yg + C0) * yg) * Src1 * C1,
        reference=lambda in0, in1, s0, s1, imm2: (
            lambda y: (y + s0) * y * in1 * s1)(
                np.square(np.asarray(in0, np.float32))),
    )
    geval_op = mk("KA_GEVAL", geval_spec)

    # out = (in0 + C0) * in1   (k-stream: (o_k + bg2) * o_q)   2 stages
    kam_spec = Spec(
        body=(Src0 + C0) * Src1,
        reference=lambda in0, in1, s0, s1, imm2: (
            (np.asarray(in0, np.float32) + s0) * in1),
    )
    kam_op = mk("KA_ADDMUL", kam_spec)
    return recip_op, recip2_op, geval_op, kam_op


def _build_nc(reps=1):
    from contextlib import ExitStack
    import concourse.bass as bass
    import concourse.tile as tile
    import concourse.mybir as mybir
    from concourse import bacc

    deval_op, recip2_op, geval_op, kam_op = _register_ops()

    dt = mybir.dt
    f16 = dt.float16
    f32 = dt.float32
    AF = mybir.ActivationFunctionType

    nc = bacc.Bacc(
        "TRN2",
        target_bir_lowering=False,
        debug=False,
        enable_asserts=False,
        num_devices=NCORES,
    )

    ins = {}
    def din(name, shape, dty):
        ins[name] = nc.dram_tensor(name, shape, dty, kind="ExternalInput").ap()
        return ins[name]

    xq = din("xq", [G * (D + 1), BC], f16)      # rows g*65+d (d<64: x^T), row 64: ones
    xk = din("xk", [G * (D + 1), BC], f16)
    # rows g*65+d: [W1[g,d,:] (512) | bypass W1c@W2c/2 (64)], row 64: biases
    w1q = din("w1q", [G * (D + 1), H + P], f16)
    w1k = din("w1k", [G * (D + 1), H + P], f16)
    # pair-packed W2: row j*128+r, col s*256 + hc*64 + p = W2[2j+s, hc*128+r, p]
    w2q = din("w2q", [NPAIR * 128, 512], f16)
    w2k = din("w2k", [NPAIR * 128, 512], f16)
    # rows 0-63: [Wg1 (512) | bypass Wg1c@Wg2c/2 (32)], row 64: bg1 / bypass bias
    wg1 = din("wg1", [D + 1, H + 32], f16)
    wg2 = din("wg2", [128, 4 * 32], f16)        # [r, hc*32+e] = Wg2[hc*128+r, e] (e<16, else 0)
    b2q = din("b2q", [64, G], f32)              # col g = b2[g]
    b2k = din("b2k", [64, G], f32)
    bg2r = din("bg2r", [128, 1], f32)           # 4x [bg2(16); zeros(16)] along partitions
    ones128 = din("ones128", [128, 1], f16)
    onesbc = din("onesbc", [1, BC], f16)

    out = nc.dram_tensor("out", [1, BC], f32, kind="ExternalOutput").ap()

    with tile.TileContext(nc) as tc:
        with ExitStack() as ctx:
            ep = ctx.enter_context
            px = ep(tc.tile_pool(name="px", bufs=8))          # x tiles [65,BC]
            pw1 = ep(tc.tile_pool(name="pw1", bufs=8))        # W1 tiles [65,H+P]
            pw2 = ep(tc.tile_pool(name="pw2", bufs=3))        # W2 pair tiles [128,512]
            phs = ep(tc.tile_pool(name="phs", bufs=8))        # silu'd h [128,512]
            pyr = ep(tc.tile_pool(name="pyr", bufs=4))        # y and r tiles [128,512]
            pfv = ep(tc.tile_pool(name="pfv", bufs=3))        # f pair tiles [128,BC]
            pu = ep(tc.tile_pool(name="pu", bufs=6))          # U tiles [128,BC]
            ph2 = ep(tc.tile_pool(name="ph2", bufs=10))       # silu'd h2 [128,512]
            pbig = ep(tc.tile_pool(name="pbig", bufs=2))      # qs/ks big [128,16*BC]
            pmisc = ep(tc.tile_pool(name="pmisc", bufs=2))
            pconst = ep(tc.tile_pool(name="pconst", bufs=1))
            # PSUM: psh 3 + psc 2 (chunk-3 tiles) + psv 3 = 8 banks
            psh = ep(tc.tile_pool(name="psh", bufs=3, space="PSUM"))
            psc = ep(tc.tile_pool(name="psc", bufs=2, space="PSUM"))
            psv = ep(tc.tile_pool(name="psv", bufs=3, space="PSUM"))
            pdram = ep(tc.tile_pool(name="pdram", bufs=4, space="DRAM"))

            def const_tile(src_ap, shape, dty, name):
                t = pconst.tile(shape, dty, name=name, tag=name)
                nc.sync.dma_start(t[:, :], src_ap)
                return t

            wg1_s = const_tile(wg1, [D + 1, H + 32], f16, "wg1s")
            wg2_s = const_tile(wg2, [128, 4 * 32], f16, "wg2s")
            b2q_s = const_tile(b2q, [64, G], f32, "b2qs")
            b2k_s = const_tile(b2k, [64, G], f32, "b2ks")
            bg2_s = const_tile(bg2r, [128, 1], f32, "bg2s")
            one_s = const_tile(ones128, [128, 1], f16, "ones")

            stream_in = {"q": (xq, w1q, w2q, b2q_s), "k": (xk, w1k, w2k, b2k_s)}

            def dve_silu_chunk(z_sl, fit):
                """3-instr DVE path: g(z) for one [128,512] PSUM slice."""
                n1, c0, d1, d0, cseed = fit
                d_t = pyr.tile([128, 512], f16, tag="y")
                nc.vector._custom_dve(deval_op, out=d_t[:, :], in0=z_sl,
                                      s0=d1, s1=d0)
                r_t = pyr.tile([128, 512], f16, tag="r")
                nc.vector._custom_dve(recip2_op, out=r_t[:, :], in0=d_t[:, :],
                                      s0=cseed)
                g_t = phs.tile([128, 512], f16, tag="hs")
                nc.vector._custom_dve(geval_op, out=g_t[:, :], in0=z_sl,
                                      in1=r_t[:, :], s0=c0, s1=n1)
                return g_t

            # ================= grouped stage =================
            # Software-pipelined: chunk 3 (the DVE-offloadable one) gets its
            # GEMM1 first into the dedicated psc pool and its silu result is
            # consumed one group LATE (GEMM2 c3 + eviction close in the next
            # group's iteration), so no engine ever stalls on the 3-instr DVE
            # chain's latency.
            def grouped(s, fd):
                x_d, w1_d, w2_d, b2_s = stream_in[s]
                carry = [None]   # (v_ps, w2slice_c3, hs3, fv, b2ap, sgi, j)
                fv_done = []     # pair js whose fv got both evictions

                def close_carry():
                    if carry[0] is None:
                        return
                    v_ps_, w2c3, hs3, fv_, b2ap, sgi_, j_ = carry[0]
                    nc.tensor.matmul(v_ps_[:, :], w2c3, hs3[:, :],
                                     start=False, stop=True)
                    nc.vector.tensor_scalar_add(
                        fv_[sgi_ * 64:(sgi_ + 1) * 64, :], v_ps_[:, :], b2ap)
                    if sgi_ == 1:
                        nc.gpsimd.dma_start(
                            fd[j_ * 128:(j_ + 1) * 128, :], fv_[:, :])
                    carry[0] = None

                fv = None
                for g in range(G):
                    j, sgi = divmod(g, 2)
                    off = _offsel(g, OFF_G)
                    if sgi == 0:
                        w2_t = pw2.tile([128, 512], f16, tag="w2")
                        nc.gpsimd.dma_start(
                            w2_t[:, :], w2_d[j * 128:(j + 1) * 128, :])
                        fv = pfv.tile([128, BC], f16, tag="fv")
                    x_t = px.tile([D + 1, BC], f16, tag="x")
                    nc.sync.dma_start(x_t[:, :], x_d[g * 65:(g + 1) * 65, :])
                    w1_t = pw1.tile([D + 1, H + P], f16, tag="w1")
                    nc.sync.dma_start(w1_t[:, :], w1_d[g * 65:(g + 1) * 65, :])
                    v_ps = psv.tile([64, BC], f32, tag="vps")
                    # chunk 3 first, into the dedicated pool
                    hp3 = psc.tile([128, 512], f32, tag="hc3")
                    nc.tensor.matmul(hp3[:, :], w1_t[:, 3 * 128:4 * 128],
                                     x_t[:, :], start=True, stop=True)
                    if off:
                        hs3 = dve_silu_chunk(hp3[:, :], FIT1)
                    else:
                        hs3 = phs.tile([128, 512], f16, tag="hs")
                        nc.scalar.activation(hs3[:, :], hp3[:, :], AF.Silu)
                    # close the PREVIOUS group (its hs3 is ready by now)
                    close_carry()
                    for hc in range(3):
                        hp = psh.tile([128, 512], f32, tag="hps")
                        nc.tensor.matmul(hp[:, :],
                                         w1_t[:, hc * 128:(hc + 1) * 128],
                                         x_t[:, :], start=True, stop=True)
                        hs_t = phs.tile([128, 512], f16, tag="hs")
                        nc.scalar.activation(hs_t[:, :], hp[:, :], AF.Silu)
                        nc.tensor.matmul(
                            v_ps[:, :],
                            w2_t[:, sgi * 256 + hc * 64:sgi * 256 + (hc + 1) * 64],
                            hs_t[:, :],
                            start=(hc == 0), stop=False,
                        )
                    if off:   # z/2 bypass for the offloaded chunk
                        nc.tensor.matmul(v_ps[:, :], w1_t[:, H:H + P],
                                         x_t[:, :], start=False, stop=False)
                    carry[0] = (v_ps,
                                w2_t[:, sgi * 256 + 3 * 64:sgi * 256 + 4 * 64],
                                hs3, fv, b2_s[:, g:g + 1], sgi, j)
                close_carry()

            # ================= global stage =================
            # per patch p: u_p [65, BC] (64 groups + ones row via DMA), bg1
            # rides as wg1_s row 64 so the 512-wide acts are bias-free
            def global_stream(s, fd, big, qbig=None):
                fd3 = fd.rearrange("(g p) b -> p g b", p=P)
                pending = []
                def evict(o_ps_, p__):
                    pr, pcb = 32 * (p__ % 4), (p__ // 4) * BC
                    if qbig is None:
                        nc.vector.tensor_scalar_add(
                            big[pr:pr + 32, pcb:pcb + BC], o_ps_[:, :],
                            bg2_s[pr:pr + 32, 0:1])
                    elif USE_KAM:   # fused (o_k + bg2) * o_q into the k big buffer
                        nc.vector._custom_dve(
                            kam_op,
                            out=big[pr:pr + 32, pcb:pcb + BC],
                            in0=o_ps_[:, :],
                            in1=qbig[pr:pr + 32, pcb:pcb + BC],
                            s0=bg2_s[pr:pr + 32, 0:1])
                    else:
                        nc.vector.tensor_scalar_add(
                            big[pr:pr + 32, pcb:pcb + BC], o_ps_[:, :],
                            bg2_s[pr:pr + 32, 0:1])
                        nc.vector.tensor_mul(
                            big[pr:pr + 32, pcb:pcb + BC],
                            qbig[pr:pr + 32, pcb:pcb + BC],
                            big[pr:pr + 32, pcb:pcb + BC])
                carry = [None]   # (o_ps, hs3, p_)
                def close_carry():
                    if carry[0] is None:
                        return
                    o_ps_, hs3_, p__ = carry[0]
                    nc.tensor.matmul(o_ps_[:, :], wg2_s[:, 3 * 32:4 * 32],
                                     hs3_[:, :], start=False, stop=True)
                    evict(o_ps_, p__)
                    carry[0] = None
                for p_ in range(P):
                    offp = _offsel(p_, OFF_P)
                    u_t = pu.tile([D + 1, BC], f16, tag="u")
                    nc.sync.dma_start(u_t[0:D, :], fd3[p_])
                    nc.sync.dma_start(u_t[D:D + 1, :], onesbc)
                    hp3 = psc.tile([128, 512], f32, tag="hc3")
                    nc.tensor.matmul(hp3[:, :], wg1_s[:, 3 * 128:4 * 128],
                                     u_t[:, :], start=True, stop=True)
                    if offp:
                        hs3 = dve_silu_chunk(hp3[:, :], FIT2)
                    else:
                        hs3 = ph2.tile([128, 512], f16, tag="h2s")
                        nc.scalar.activation(hs3[:, :], hp3[:, :], AF.Silu)
                    close_carry()
                    o_ps = psv.tile([32, BC], f32, tag="vps")
                    for hc in range(3):
                        h2p = psh.tile([128, 512], f32, tag="hps")
                        nc.tensor.matmul(
                            h2p[:, :],
                            wg1_s[:, hc * 128:(hc + 1) * 128],
                            u_t[:, :],
                            start=True, stop=True,
                        )
                        ht = ph2.tile([128, 512], f16, tag="h2s")
                        nc.scalar.activation(ht[:, :], h2p[:, :], AF.Silu)
                        nc.tensor.matmul(
                            o_ps[:, :],
                            wg2_s[:, hc * 32:(hc + 1) * 32],
                            ht[:, :],
                            start=(hc == 0), stop=False,
                        )
                    if offp:   # z/2 bypass for the offloaded chunk
                        nc.tensor.matmul(
                            o_ps[:, :],
                            wg1_s[:, H:H + 32],
                            u_t[:, :],
                            start=False, stop=False,
                        )
                    carry[0] = (o_ps, hs3, p_)
                close_carry()

            for _ in range(reps):
                f_q = pdram.tile([G * P, BC], f16, tag="fq")
                f_k = pdram.tile([G * P, BC], f16, tag="fk")
                grouped("q", f_q)
                grouped("k", f_k)

                qs_big = pbig.tile([128, 16 * BC], f16, tag="qsbig")
                ks_big = pbig.tile([128, 16 * BC], f16, tag="ksbig")
                global_stream("q", f_q, qs_big)
                global_stream("k", f_k, ks_big, qbig=qs_big)

                # ===== logits: reduce q*k product with accumulating matmuls =====
                at_ps = psv.tile([1, BC], f32, tag="vps")
                for c in range(16):
                    nc.tensor.matmul(at_ps[0:1, :], one_s[:, 0:1],
                                     ks_big[:, c * BC:(c + 1) * BC],
                                     start=(c == 0), stop=(c == 15))
                at_s = pmisc.tile([1, BC], f32, tag="at")
                nc.vector.tensor_copy(at_s[0:1, :], at_ps[0:1, :])
                nc.sync.dma_start(out[0:1, :], at_s[0:1, :])

    nc.compile()
    return nc


_NC_CACHE = {}


def _get_nc(reps=1):
    if reps not in _NC_CACHE:
        _NC_CACHE[reps] = _build_nc(reps)
    return _NC_CACHE[reps]


def _prep_inputs(q, k, W1q, b1q, W2q, b2q, W1k, b1k, W2k, b2k, Wg1, bg1, Wg2, bg2):
    f16c = lambda a: np.ascontiguousarray(a, dtype=np.float16)
    f32c = lambda a: np.ascontiguousarray(a, dtype=np.float32)

    def pack_x(x):  # [B, 4096] -> per-core [G*65, BC] with ones row
        shards = []
        for c in range(NCORES):
            xs = np.asarray(x[c * BC:(c + 1) * BC, :], np.float32)
            xt = np.empty((G, D + 1, BC), dtype=np.float16)
            xt[:, :D, :] = xs.T.reshape(G, D, BC).astype(np.float16)
            xt[:, D, :] = 1.0
            shards.append(np.ascontiguousarray(xt.reshape(G * (D + 1), BC)))
        return shards

    def pack_w1(W1, b1, W2):
        W1 = np.asarray(W1, np.float32)
        b1 = np.asarray(b1, np.float32)
        W2 = np.asarray(W2, np.float32)
        w = np.empty((G, D + 1, H + P), dtype=np.float32)
        w[:, :D, :H] = W1
        w[:, D, :H] = b1
        # bypass: z/2 @ W2 for hidden chunk 3 (384:512)
        W1c = W1[:, :, 384:512]            # [G, 64, 128]
        W2c = W2[:, 384:512, :]            # [G, 128, 64]
        w[:, :D, H:] = 0.5 * np.einsum('gdj,gjp->gdp', W1c, W2c)
        w[:, D, H:] = 0.5 * np.einsum('gj,gjp->gp', b1[:, 384:512], W2c)
        return f16c(w.reshape(G * (D + 1), H + P))

    def pack_w2(W2):  # [G, 512, 64] -> [G/2*128, 512] pair-packed
        w = np.asarray(W2, np.float32).reshape(G, 4, 128, 64)   # [g, hc, r, p]
        w = w.transpose(0, 2, 1, 3).reshape(G, 128, 256)        # [g, r, hc*64+p]
        w = w.reshape(G // 2, 2, 128, 256).transpose(0, 2, 1, 3)  # [j, r, s, :]
        return f16c(w.reshape(NPAIR * 128, 512))

    xq_s = pack_x(q)
    xk_s = pack_x(k)
    w1q_p = pack_w1(W1q, b1q, W2q)
    w1k_p = pack_w1(W1k, b1k, W2k)
    w2q_p = pack_w2(W2q)
    w2k_p = pack_w2(W2k)
    b2q_p = f32c(np.asarray(b2q, np.float32).T)                 # [64(P), G]
    b2k_p = f32c(np.asarray(b2k, np.float32).T)

    Wg1f = np.asarray(Wg1, np.float32)
    bg1f = np.asarray(bg1, np.float32)
    Wg2f = np.asarray(Wg2, np.float32)
    wg1_p = np.zeros((D + 1, H + 32), dtype=np.float32)
    wg1_p[:D, :H] = Wg1f
    wg1_p[D, :H] = bg1f
    wg1_p[:D, H:H + E] = 0.5 * (Wg1f[:, 384:512] @ Wg2f[384:512, :])
    wg1_p[D, H:H + E] = 0.5 * (bg1f[384:512] @ Wg2f[384:512, :])
    wg1_p = f16c(wg1_p)

    wg2_p = np.zeros((128, 4, 32), dtype=np.float16)
    wg2_p[:, :, :E] = Wg2f.reshape(4, 128, E).transpose(1, 0, 2).astype(np.float16)
    wg2_p = np.ascontiguousarray(wg2_p.reshape(128, 4 * 32))    # [r, hc*32+e]
    bg2_p = np.zeros((4, 32), dtype=np.float32)
    bg2_p[:, :E] = np.asarray(bg2, np.float32)
    bg2_p = f32c(bg2_p.reshape(128, 1))
    ones_p = np.ones((128, 1), dtype=np.float16)

    in_maps = []
    for c in range(NCORES):
        in_maps.append({
            "xq": xq_s[c], "xk": xk_s[c],
            "w1q": w1q_p, "w1k": w1k_p,
            "w2q": w2q_p, "w2k": w2k_p,
            "wg1": wg1_p, "wg2": wg2_p,
            "b2q": b2q_p, "b2k": b2k_p,
            "bg2r": bg2_p, "ones128": ones_p,
            "onesbc": np.ones((1, BC), dtype=np.float16),
        })
    return in_maps


def kernel(q, k, W1q, b1q, W2q, b2q, W1k, b1k, W2k, b2k, Wg1, bg1, Wg2, bg2,
           _trace=False, _tracedir=None):
    from concourse.bass_utils import run_bass_kernel_spmd

    in_maps = _prep_inputs(q, k, W1q, b1q, W2q, b2q, W1k, b1k, W2k, b2k,
                           Wg1, bg1, Wg2, bg2)
    nc = _get_nc()
    kw = {}
    if _trace:
        kw = dict(trace=True, tmpdir=_tracedir)
    res = run_bass_kernel_spmd(nc, in_maps, core_ids=list(range(NCORES)), **kw)
    logits = np.concatenate([res.results[c]["out"].reshape(BC)
                             for c in range(NCORES)]).astype(np.float64)
    m = logits.max()
    e = np.exp(logits - m)
    sm = (e / e.sum()).astype(np.float32)
    if _trace:
        kernel._last_trace = res
    return sm


# revision 23
# speedup vs baseline: 1.0935x; 1.0935x over previous
"""Trainium2 Bass kernel for GroupedKAAttention (fp16 datapath, v4).

Math per batch row b (B=4096 total, 512 per core, data-parallel over 8
NeuronCores, weights replicated):
  xg[b,g,:]  = x[b, g*64:(g+1)*64]                      (G=64 groups, D=64)
  h[b,g,:]   = silu(xg[b,g,:] @ W1[g] + b1[g])          (H=512)
  f[b,g,:]   = h[b,g,:] @ W2[g] + b2[g]                 (P=64 patches)
  h2[b,p,:]  = silu(f[b,:,p] @ Wg1 + bg1)               (contract groups)
  o[b,p,:]   = h2[b,p,:] @ Wg2 + bg2                    (E=16 heads)
  attn[b]    = sum_{p,e} o_q * o_k ;  out = softmax(attn over b)

v4: the scalar (Act) engine is the roofline (~67M silu evals/core, 98%
busy in v3).  A slice of the silu work moves to the idle DVE via two
runtime-registered custom DVE ops, using the identity
  silu(z) = z/2 + g(z),   g(z) = (z/2)tanh(z/2)  (EVEN in z)
g is approximated by a (2,2) rational in y=z^2 (weighted rms ~1e-5):
  g ~= n1*y*(y+c0) / (y^2 + d1*y + d0)
For an offloaded hidden-chunk (128 of 512 hidden units):
  1. y  = z*z                    stock tensor_tensor from PSUM  (658ns)
  2. r  ~= 1/D(y)                custom op: monic quadratic, bitwise-NOT
                                 reciprocal seed + 1 Newton step (8 ALU
                                 stages, 593ns)
  3. g  = n1*(y+c0)*y*r          custom op (4 stages, 593ns)
The missing linear z/2 rides the next GEMM exactly: the stationary W1/Wg1
tiles carry 64/32 extra columns holding W1c@W2c/2 (resp Wg1c@Wg2c/2) so
one extra accumulating matmul per offloaded group/patch adds z/2 @ W2.
OFF_G/OFF_P control how many groups/patches offload chunk 3 (balance Act
vs DVE).  End-to-end approx error at full offload of chunk3 everywhere
measured 5.4e-3 (gate 2e-2); at OFF_G=OFF_P=46 it is ~2-3e-3.

Other v4 changes: the u-tile ones row comes from a DMA (not a DVE copy);
the k-stream bias-add + q*k product is one fused custom DVE op.

`reps` unrolls the computation R times inside one NEFF (weights stay
SBUF-resident, f bounce double-buffered) for steady-state throughput
benchmarking; the correctness path uses reps=1.
"""

import numpy as np

B = 4096
TOTAL_DIM = 4096
G = 64            # groups
D = 64            # group size
H = 512           # hidden
P = 64            # patches
E = 16            # heads
NCORES = 8
BC = B // NCORES  # 512 batch rows per core
NPAIR = P // 2    # 32 patch pairs (global stage)

# how many of the 64 groups (grouped stage) / 64 patches (global stage)
# route hidden-chunk 3 through the DVE instead of the Act engine
OFF_G = 46
OFF_P = 46
# KA_ADDMUL with partition-shifted operands produced NaN on HW; the stock
# add+mul pair costs ~21us more DVE but is correct.
USE_KAM = False

# rational-fit constants per stage: (n1, c0, d1, d0, seed_const)
FIT1 = (11.37246959, 44.892305, 215.49627357, 2042.31513025, -0.235292)
FIT2 = (11.56992132, 44.93750663, 219.51844636, 2079.73021591, -0.235293)


def _offsel(i, n_off, n_tot=64):
    return (i * n_off) % n_tot < n_off


def _register_ops():
    """Register the custom DVE ops (idempotent across rebuilds)."""
    from concourse import dve_ops
    from concourse.dve_spec import (
        Spec, Src0, Src1, C0, C1, C2, Bin, AluOp, Zero, One, lower, _has_src1)
    from concourse.dve_uop import DveOpSpec

    def mk(name, spec):
        for o in dve_ops.OPS:
            if o.name == name:
                return o
        row = dve_ops._CUSTOM_DVE_ROW_BASE + len(dve_ops.OPS)
        dve_ops._SUB_OPCODE_FOR_NAME[name] = row
        shas = {}
        for ver in ("v3", "v4"):
            s = DveOpSpec(name=name, opcode=row, uops=lower(spec, ver=ver),
                          rd1_en=_has_src1(spec))
            shas[ver] = s.sha(ver)
        op = dve_ops.DveOp(name, spec, subdim=False, uops_sha=shas)
        dve_ops.OPS.append(op)
        dve_ops.CUSTOM_DVE_SPECS[name] = spec
        return op

    def _np_not(a):
        return (~np.asarray(a, np.float32).view(np.int32)).view(np.float32)

    from concourse.dve_spec import sq

    # D = (z^2 + C0)*z^2 + C1  — single PSUM read (PSUM has one DVE port)
    yq = sq(Src0)
    deval_spec = Spec(
        body=(yq + C0) * yq + C1,
        reference=lambda in0, s0, s1, imm2: (
            lambda y: (y + s0) * y + s1)(np.square(np.asarray(in0, np.float32))),
    )
    recip_op = mk("KA_DEVAL", deval_spec)

    # r ~= 1/D: bitwise-NOT seed * C0, then two Newton steps. 8 ALU stages.
    seed = Bin(AluOp.BITWISE_NOT, Src0, Src0) * C0
    r1 = seed * ((One + One) - Src0 * seed)
    recip2_spec = Spec(
        body=r1 * ((One + One) - Src0 * r1),
        reference=lambda in0, s0, s1, imm2: (
            lambda Dv: (lambda s: (lambda a: a * (2.0 - Dv * a))(
                s * (2.0 - Dv * s)))(_np_not(Dv) * s0)
        )(np.asarray(in0, np.float32)),
    )
    recip2_op = mk("KA_RECIP2", recip2_spec)

    # g = ((z^2+C0)*z^2) * r * C1    (5 stages; z from PSUM, r elementwise)
    yg = sq(Src0)
    geval_spec = Spec(
        body=((yg + C0) * yg) * Src1 * C1,
        reference=lambda in0, in1, s0, s1, imm2: (
            lambda y: (y + s0) * y * in1 * s1)(
                np.square(np.asarray(in0, np.float32))),
    )
    geval_op = mk("KA_GEVAL", geval_spec)

    # out = (in0 + C0) * in1   (k-stream: (o_k + bg2) * o_q)   2 stages
    kam_spec = Spec(
        body=(Src0 + C0) * Src1,
        reference=lambda in0, in1, s0, s1, imm2: (
            (np.asarray(in0, np.float32) + s0) * in1),
    )
    kam_op = mk("KA_ADDMUL", kam_spec)
    return recip_op, recip2_op, geval_op, kam_op


def _build_nc(reps=1):
    from contextlib import ExitStack
    import concourse.bass as bass
    import concourse.tile as tile
    import concourse.mybir as mybir
    from concourse import bacc

    deval_op, recip2_op, geval_op, kam_op = _register_ops()

    dt = mybir.dt
    f16 = dt.float16
    f32 = dt.float32
    AF = mybir.ActivationFunctionType

    nc = bacc.Bacc(
        "TRN2",
        target_bir_lowering=False,
        debug=False,
        enable_asserts=False,
        num_devices=NCORES,
    )

    ins = {}
    def din(name, shape, dty):
        ins[name] = nc.dram_tensor(name, shape, dty, kind="ExternalInput").ap()
        return ins[name]

    xq = din("xq", [G * (D + 1), BC], f16)      # rows g*65+d (d<64: x^T), row 64: ones
    xk = din("xk", [G * (D + 1), BC], f16)
    # rows g*65+d: [W1[g,d,:] (512) | bypass W1c@W2c/2 (64)], row 64: biases
    w1q = din("w1q", [G * (D + 1), H + P], f16)
    w1k = din("w1k", [G * (D + 1), H + P], f16)
    # pair-packed W2: row j*128+r, col s*256 + hc*64 + p = W2[2j+s, hc*128+r, p]
    w2q = din("w2q", [NPAIR * 128, 512], f16)
    w2k = din("w2k", [NPAIR * 128, 512], f16)
    # rows 0-63: [Wg1 (512) | bypass Wg1c@Wg2c/2 (32)], row 64: bg1 / bypass bias
    wg1 = din("wg1", [D + 1, H + 32], f16)
    wg2 = din("wg2", [128, 4 * 32], f16)        # [r, hc*32+e] = Wg2[hc*128+r, e] (e<16, else 0)
    b2q = din("b2q", [64, G], f32)              # col g = b2[g]
    b2k = din("b2k", [64, G], f32)
    bg2r = din("bg2r", [128, 1], f32)           # 4x [bg2(16); zeros(16)] along partitions
    ones128 = din("ones128", [128, 1], f16)
    onesbc = din("onesbc", [1, BC], f16)

    out = nc.dram_tensor("out", [1, BC], f32, kind="ExternalOutput").ap()

    with tile.TileContext(nc) as tc:
        with ExitStack() as ctx:
            ep = ctx.enter_context
            px = ep(tc.tile_pool(name="px", bufs=8))          # x tiles [65,BC]
            pw1 = ep(tc.tile_pool(name="pw1", bufs=8))        # W1 tiles [65,H+P]
            pw2 = ep(tc.tile_pool(name="pw2", bufs=3))        # W2 pair tiles [128,512]
            phs = ep(tc.tile_pool(name="phs", bufs=8))        # silu'd h [128,512]
            pyr = ep(tc.tile_pool(name="pyr", bufs=4))        # y and r tiles [128,512]
            pfv = ep(tc.tile_pool(name="pfv", bufs=3))        # f pair tiles [128,BC]
            pu = ep(tc.tile_pool(name="pu", bufs=6))          # U tiles [128,BC]
            ph2 = ep(tc.tile_pool(name="ph2", bufs=10))       # silu'd h2 [128,512]
            pbig = ep(tc.tile_pool(name="pbig", bufs=2))      # qs/ks big [128,16*BC]
            pmisc = ep(tc.tile_pool(name="pmisc", bufs=2))
            pconst = ep(tc.tile_pool(name="pconst", bufs=1))
            # PSUM: psh 3 + psc 2 (chunk-3 tiles) + psv 3 = 8 banks
            psh = ep(tc.tile_pool(name="psh", bufs=3, space="PSUM"))
            psc = ep(tc.tile_pool(name="psc", bufs=2, space="PSUM"))
            psv = ep(tc.tile_pool(name="psv", bufs=3, space="PSUM"))
            pdram = ep(tc.tile_pool(name="pdram", bufs=4, space="DRAM"))

            def const_tile(src_ap, shape, dty, name):
                t = pconst.tile(shape, dty, name=name, tag=name)
                nc.sync.dma_start(t[:, :], src_ap)
                return t

            wg1_s = const_tile(wg1, [D + 1, H + 32], f16, "wg1s")
            wg2_s = const_tile(wg2, [128, 4 * 32], f16, "wg2s")
            b2q_s = const_tile(b2q, [64, G], f32, "b2qs")
            b2k_s = const_tile(b2k, [64, G], f32, "b2ks")
            bg2_s = const_tile(bg2r, [128, 1], f32, "bg2s")
            one_s = const_tile(ones128, [128, 1], f16, "ones")
            ones_bc = const_tile(onesbc, [1, BC], f16, "onesbc")

            stream_in = {"q": (xq, w1q, w2q, b2q_s), "k": (xk, w1k, w2k, b2k_s)}

            def dve_silu_chunk(z_sl, fit):
                """3-instr DVE path: g(z) for one [128,512] PSUM slice."""
                n1, c0, d1, d0, cseed = fit
                d_t = pyr.tile([128, 512], f16, tag="y")
                nc.vector._custom_dve(deval_op, out=d_t[:, :], in0=z_sl,
                                      s0=d1, s1=d0)
                r_t = pyr.tile([128, 512], f16, tag="r")
                nc.vector._custom_dve(recip2_op, out=r_t[:, :], in0=d_t[:, :],
                                      s0=cseed)
                g_t = phs.tile([128, 512], f16, tag="hs")
                nc.vector._custom_dve(geval_op, out=g_t[:, :], in0=z_sl,
                                      in1=r_t[:, :], s0=c0, s1=n1)
                return g_t

            # ================= grouped stage =================
            # Software-pipelined: chunk 3 (the DVE-offloadable one) gets its
            # GEMM1 first into the dedicated psc pool and its silu result is
            # consumed one group LATE (GEMM2 c3 + eviction close in the next
            # group's iteration), so no engine ever stalls on the 3-instr DVE
            # chain's latency.
            def grouped(s, fd):
                x_d, w1_d, w2_d, b2_s = stream_in[s]
                carry = [None]   # (v_ps, w2slice_c3, hs3, fv, b2ap, sgi, j)
                fv_done = []     # pair js whose fv got both evictions

                def close_carry():
                    if carry[0] is None:
                        return
                    v_ps_, w2c3, hs3, fv_, b2ap, sgi_, j_ = carry[0]
                    nc.tensor.matmul(v_ps_[:, :], w2c3, hs3[:, :],
                                     start=False, stop=True)
                    nc.vector.tensor_scalar_add(
                        fv_[sgi_ * 64:(sgi_ + 1) * 64, :], v_ps_[:, :], b2ap)
                    if sgi_ == 1:
                        nc.gpsimd.dma_start(
                            fd[j_ * 128:(j_ + 1) * 128, :], fv_[:, :])
                    carry[0] = None

                fv = None
                for g in range(G):
                    j, sgi = divmod(g, 2)
                    off = _offsel(g, OFF_G)
                    if sgi == 0:
                        w2_t = pw2.tile([128, 512], f16, tag="w2")
                        nc.gpsimd.dma_start(
                            w2_t[:, :], w2_d[j * 128:(j + 1) * 128, :])
                        fv = pfv.tile([128, BC], f16, tag="fv")
                    x_t = px.tile([D + 1, BC], f16, tag="x")
                    nc.sync.dma_start(x_t[:, :], x_d[g * 65:(g + 1) * 65, :])
                    w1_t = pw1.tile([D + 1, H + P], f16, tag="w1")
                    nc.sync.dma_start(w1_t[:, :], w1_d[g * 65:(g + 1) * 65, :])
                    v_ps = psv.tile([64, BC], f32, tag="vps")
                    # chunk 3 first, into the dedicated pool
                    hp3 = psc.tile([128, 512], f32, tag="hc3")
                    nc.tensor.matmul(hp3[:, :], w1_t[:, 3 * 128:4 * 128],
                                     x_t[:, :], start=True, stop=True)
                    if off:
                        hs3 = dve_silu_chunk(hp3[:, :], FIT1)
                    else:
                        hs3 = phs.tile([128, 512], f16, tag="hs")
                        nc.scalar.activation(hs3[:, :], hp3[:, :], AF.Silu)
                    # close the PREVIOUS group (its hs3 is ready by now)
                    close_carry()
                    for hc in range(3):
                        hp = psh.tile([128, 512], f32, tag="hps")
                        nc.tensor.matmul(hp[:, :],
                                         w1_t[:, hc * 128:(hc + 1) * 128],
                                         x_t[:, :], start=True, stop=True)
                        hs_t = phs.tile([128, 512], f16, tag="hs")
                        nc.scalar.activation(hs_t[:, :], hp[:, :], AF.Silu)
                        nc.tensor.matmul(
                            v_ps[:, :],
                            w2_t[:, sgi * 256 + hc * 64:sgi * 256 + (hc + 1) * 64],
                            hs_t[:, :],
                            start=(hc == 0), stop=False,
                        )
                    if off:   # z/2 bypass for the offloaded chunk
                        nc.tensor.matmul(v_ps[:, :], w1_t[:, H:H + P],
                                         x_t[:, :], start=False, stop=False)
                    carry[0] = (v_ps,
                                w2_t[:, sgi * 256 + 3 * 64:sgi * 256 + 4 * 64],
                                hs3, fv, b2_s[:, g:g + 1], sgi, j)
                close_carry()

            # ================= global stage =================
            # per patch p: u_p [65, BC] (64 groups + ones row via DMA), bg1
            # rides as wg1_s row 64 so the 512-wide acts are bias-free
            def global_stream(s, fd, big, qbig=None):
                fd3 = fd.rearrange("(g p) b -> p g b", p=P)
                pending = []
                def evict(o_ps_, p__):
                    pr, pcb = 32 * (p__ % 4), (p__ // 4) * BC
                    if qbig is None:
                        nc.vector.tensor_scalar_add(
                            big[pr:pr + 32, pcb:pcb + BC], o_ps_[:, :],
                            bg2_s[pr:pr + 32, 0:1])
                    elif USE_KAM:   # fused (o_k + bg2) * o_q into the k big buffer
                        nc.vector._custom_dve(
                            kam_op,
                            out=big[pr:pr + 32, pcb:pcb + BC],
                            in0=o_ps_[:, :],
                            in1=qbig[pr:pr + 32, pcb:pcb + BC],
                            s0=bg2_s[pr:pr + 32, 0:1])
                    else:
                        nc.vector.tensor_scalar_add(
                            big[pr:pr + 32, pcb:pcb + BC], o_ps_[:, :],
                            bg2_s[pr:pr + 32, 0:1])
                        nc.vector.tensor_mul(
                            big[pr:pr + 32, pcb:pcb + BC],
                            qbig[pr:pr + 32, pcb:pcb + BC],
                            big[pr:pr + 32, pcb:pcb + BC])
                carry = [None]   # (o_ps, hs3, p_)
                def close_carry():
                    if carry[0] is None:
                        return
                    o_ps_, hs3_, p__ = carry[0]
                    nc.tensor.matmul(o_ps_[:, :], wg2_s[:, 3 * 32:4 * 32],
                                     hs3_[:, :], start=False, stop=True)
                    evict(o_ps_, p__)
                    carry[0] = None
                for p_ in range(P):
                    offp = _offsel(p_, OFF_P)
                    u_t = pu.tile([D + 1, BC], f16, tag="u")
                    nc.sync.dma_start(u_t[0:D, :], fd3[p_])
                    nc.vector.tensor_copy(u_t[D:D + 1, :], ones_bc[0:1, :])
                    hp3 = psc.tile([128, 512], f32, tag="hc3")
                    nc.tensor.matmul(hp3[:, :], wg1_s[:, 3 * 128:4 * 128],
                                     u_t[:, :], start=True, stop=True)
                    if offp:
                        hs3 = dve_silu_chunk(hp3[:, :], FIT2)
                    else:
                        hs3 = ph2.tile([128, 512], f16, tag="h2s")
                        nc.scalar.activation(hs3[:, :], hp3[:, :], AF.Silu)
                    close_carry()
                    o_ps = psv.tile([32, BC], f32, tag="vps")
                    for hc in range(3):
                        h2p = psh.tile([128, 512], f32, tag="hps")
                        nc.tensor.matmul(
                            h2p[:, :],
                            wg1_s[:, hc * 128:(hc + 1) * 128],
                            u_t[:, :],
                            start=True, stop=True,
                        )
                        ht = ph2.tile([128, 512], f16, tag="h2s")
                        nc.scalar.activation(ht[:, :], h2p[:, :], AF.Silu)
                        nc.tensor.matmul(
                            o_ps[:, :],
                            wg2_s[:, hc * 32:(hc + 1) * 32],
                            ht[:, :],
                            start=(hc == 0), stop=False,
                        )
                    if offp:   # z/2 bypass for the offloaded chunk
                        nc.tensor.matmul(
                            o_ps[:, :],
                            wg1_s[:, H:H + 32],
                            u_t[:, :],
                            start=False, stop=False,
                        )
                    carry[0] = (o_ps, hs3, p_)
                close_carry()

            for _ in range(reps):
                f_q = pdram.tile([G * P, BC], f16, tag="fq")
                f_k = pdram.tile([G * P, BC], f16, tag="fk")
                grouped("q", f_q)
                grouped("k", f_k)

                qs_big = pbig.tile([128, 16 * BC], f16, tag="qsbig")
                ks_big = pbig.tile([128, 16 * BC], f16, tag="ksbig")
                global_stream("q", f_q, qs_big)
                global_stream("k", f_k, ks_big, qbig=qs_big)

                # ===== logits: reduce q*k product with accumulating matmuls =====
                at_ps = psv.tile([1, BC], f32, tag="vps")
                for c in range(16):
                    nc.tensor.matmul(at_ps[0:1, :], one_s[:, 0:1],
                                     ks_big[:, c * BC:(c + 1) * BC],
                                     start=(c == 0), stop=(c == 15))
                at_s = pmisc.tile([1, BC], f32, tag="at")
                nc.vector.tensor_copy(at_s[0:1, :], at_ps[0:1, :])
                nc.sync.dma_start(out[0:1, :], at_s[0:1, :])

    nc.compile()
    return nc


_NC_CACHE = {}


def _get_nc(reps=1):
    if reps not in _NC_CACHE:
        _NC_CACHE[reps] = _build_nc(reps)
    return _NC_CACHE[reps]


def _prep_inputs(q, k, W1q, b1q, W2q, b2q, W1k, b1k, W2k, b2k, Wg1, bg1, Wg2, bg2):
    f16c = lambda a: np.ascontiguousarray(a, dtype=np.float16)
    f32c = lambda a: np.ascontiguousarray(a, dtype=np.float32)

    def pack_x(x):  # [B, 4096] -> per-core [G*65, BC] with ones row
        shards = []
        for c in range(NCORES):
            xs = np.asarray(x[c * BC:(c + 1) * BC, :], np.float32)
            xt = np.empty((G, D + 1, BC), dtype=np.float16)
            xt[:, :D, :] = xs.T.reshape(G, D, BC).astype(np.float16)
            xt[:, D, :] = 1.0
            shards.append(np.ascontiguousarray(xt.reshape(G * (D + 1), BC)))
        return shards

    def pack_w1(W1, b1, W2):
        W1 = np.asarray(W1, np.float32)
        b1 = np.asarray(b1, np.float32)
        W2 = np.asarray(W2, np.float32)
        w = np.empty((G, D + 1, H + P), dtype=np.float32)
        w[:, :D, :H] = W1
        w[:, D, :H] = b1
        # bypass: z/2 @ W2 for hidden chunk 3 (384:512)
        W1c = W1[:, :, 384:512]            # [G, 64, 128]
        W2c = W2[:, 384:512, :]            # [G, 128, 64]
        w[:, :D, H:] = 0.5 * np.einsum('gdj,gjp->gdp', W1c, W2c)
        w[:, D, H:] = 0.5 * np.einsum('gj,gjp->gp', b1[:, 384:512], W2c)
        return f16c(w.reshape(G * (D + 1), H + P))

    def pack_w2(W2):  # [G, 512, 64] -> [G/2*128, 512] pair-packed
        w = np.asarray(W2, np.float32).reshape(G, 4, 128, 64)   # [g, hc, r, p]
        w = w.transpose(0, 2, 1, 3).reshape(G, 128, 256)        # [g, r, hc*64+p]
        w = w.reshape(G // 2, 2, 128, 256).transpose(0, 2, 1, 3)  # [j, r, s, :]
        return f16c(w.reshape(NPAIR * 128, 512))

    xq_s = pack_x(q)
    xk_s = pack_x(k)
    w1q_p = pack_w1(W1q, b1q, W2q)
    w1k_p = pack_w1(W1k, b1k, W2k)
    w2q_p = pack_w2(W2q)
    w2k_p = pack_w2(W2k)
    b2q_p = f32c(np.asarray(b2q, np.float32).T)                 # [64(P), G]
    b2k_p = f32c(np.asarray(b2k, np.float32).T)

    Wg1f = np.asarray(Wg1, np.float32)
    bg1f = np.asarray(bg1, np.float32)
    Wg2f = np.asarray(Wg2, np.float32)
    wg1_p = np.zeros((D + 1, H + 32), dtype=np.float32)
    wg1_p[:D, :H] = Wg1f
    wg1_p[D, :H] = bg1f
    wg1_p[:D, H:H + E] = 0.5 * (Wg1f[:, 384:512] @ Wg2f[384:512, :])
    wg1_p[D, H:H + E] = 0.5 * (bg1f[384:512] @ Wg2f[384:512, :])
    wg1_p = f16c(wg1_p)

    wg2_p = np.zeros((128, 4, 32), dtype=np.float16)
    wg2_p[:, :, :E] = Wg2f.reshape(4, 128, E).transpose(1, 0, 2).astype(np.float16)
    wg2_p = np.ascontiguousarray(wg2_p.reshape(128, 4 * 32))    # [r, hc*32+e]
    bg2_p = np.zeros((4, 32), dtype=np.float32)
    bg2_p[:, :E] = np.asarray(bg2, np.float32)
    bg2_p = f32c(bg2_p.reshape(128, 1))
    ones_p = np.ones((128, 1), dtype=np.float16)

    in_maps = []
    for c in range(NCORES):
        in_maps.append({
            "xq": xq_s[c], "xk": xk_s[c],
            "w1q": w1q_p, "w1k": w1k_p,
            "w2q": w2q_p, "w2k": w2k_p,
            "wg1": wg1_p, "wg2": wg2_p,
            "b2q": b2q_p, "b2k": b2k_p,
            "bg2r": bg2_p, "ones128": ones_p,
            "onesbc": np.ones((1, BC), dtype=np.float16),
        })
    return in_maps


def kernel(q, k, W1q, b1q, W2q, b2q, W1k, b1k, W2k, b2k, Wg1, bg1, Wg2, bg2,
           _trace=False, _tracedir=None):
    from concourse.bass_utils import run_bass_kernel_spmd

    in_maps = _prep_inputs(q, k, W1q, b1q, W2q, b2q, W1k, b1k, W2k, b2k,
                           Wg1, bg1, Wg2, bg2)
    nc = _get_nc()
    kw = {}
    if _trace:
        kw = dict(trace=True, tmpdir=_tracedir)
    res = run_bass_kernel_spmd(nc, in_maps, core_ids=list(range(NCORES)), **kw)
    logits = np.concatenate([res.results[c]["out"].reshape(BC)
                             for c in range(NCORES)]).astype(np.float64)
    m = logits.max()
    e = np.exp(logits - m)
    sm = (e / e.sum()).astype(np.float32)
    if _trace:
        kernel._last_trace = res
    return sm


# revision 25
# speedup vs baseline: 1.1778x; 1.0770x over previous
"""Trainium2 Bass kernel for GroupedKAAttention (fp16 datapath, v4).

Math per batch row b (B=4096 total, 512 per core, data-parallel over 8
NeuronCores, weights replicated):
  xg[b,g,:]  = x[b, g*64:(g+1)*64]                      (G=64 groups, D=64)
  h[b,g,:]   = silu(xg[b,g,:] @ W1[g] + b1[g])          (H=512)
  f[b,g,:]   = h[b,g,:] @ W2[g] + b2[g]                 (P=64 patches)
  h2[b,p,:]  = silu(f[b,:,p] @ Wg1 + bg1)               (contract groups)
  o[b,p,:]   = h2[b,p,:] @ Wg2 + bg2                    (E=16 heads)
  attn[b]    = sum_{p,e} o_q * o_k ;  out = softmax(attn over b)

v4: the scalar (Act) engine is the roofline (~67M silu evals/core, 98%
busy in v3).  A slice of the silu work moves to the idle DVE via two
runtime-registered custom DVE ops, using the identity
  silu(z) = z/2 + g(z),   g(z) = (z/2)tanh(z/2)  (EVEN in z)
g is approximated by a (2,2) rational in y=z^2 (weighted rms ~1e-5):
  g ~= n1*y*(y+c0) / (y^2 + d1*y + d0)
For an offloaded hidden-chunk (128 of 512 hidden units):
  1. y  = z*z                    stock tensor_tensor from PSUM  (658ns)
  2. r  ~= 1/D(y)                custom op: monic quadratic, bitwise-NOT
                                 reciprocal seed + 1 Newton step (8 ALU
                                 stages, 593ns)
  3. g  = n1*(y+c0)*y*r          custom op (4 stages, 593ns)
The missing linear z/2 rides the next GEMM exactly: the stationary W1/Wg1
tiles carry 64/32 extra columns holding W1c@W2c/2 (resp Wg1c@Wg2c/2) so
one extra accumulating matmul per offloaded group/patch adds z/2 @ W2.
OFF_G/OFF_P control how many groups/patches offload chunk 3 (balance Act
vs DVE).  End-to-end approx error at full offload of chunk3 everywhere
measured 5.4e-3 (gate 2e-2); at OFF_G=OFF_P=46 it is ~2-3e-3.

Other v4 changes: the u-tile ones row comes from a DMA (not a DVE copy);
the k-stream bias-add + q*k product is one fused custom DVE op.

`reps` unrolls the computation R times inside one NEFF (weights stay
SBUF-resident, f bounce double-buffered) for steady-state throughput
benchmarking; the correctness path uses reps=1.
"""

import numpy as np

B = 4096
TOTAL_DIM = 4096
G = 64            # groups
D = 64            # group size
H = 512           # hidden
P = 64            # patches
E = 16            # heads
NCORES = 8
BC = B // NCORES  # 512 batch rows per core
NPAIR = P // 2    # 32 patch pairs (global stage)

# how many of the 64 groups (grouped stage) / 64 patches (global stage)
# route hidden-chunk 3 through the DVE instead of the Act engine
OFF_G = 46
OFF_P = 46
OFF_P_K = None   # k-stream global offload count; None = same as OFF_P
# KA_ADDMUL with partition-shifted operands produced NaN on HW; the stock
# add+mul pair costs ~21us more DVE but is correct.
USE_KAM = False

# rational-fit constants per stage: (n1, c0, d1, d0, seed_const)
FIT1 = (11.37246959, 44.892305, 215.49627357, 2042.31513025, -0.235292)
FIT2 = (11.56992132, 44.93750663, 219.51844636, 2079.73021591, -0.235293)


def _offsel(i, n_off, n_tot=64):
    return (i * n_off) % n_tot < n_off


def _register_ops():
    """Register the custom DVE ops (idempotent across rebuilds)."""
    from concourse import dve_ops
    from concourse.dve_spec import (
        Spec, Src0, Src1, C0, C1, C2, Bin, AluOp, Zero, One, lower, _has_src1)
    from concourse.dve_uop import DveOpSpec

    def mk(name, spec):
        for o in dve_ops.OPS:
            if o.name == name:
                return o
        row = dve_ops._CUSTOM_DVE_ROW_BASE + len(dve_ops.OPS)
        dve_ops._SUB_OPCODE_FOR_NAME[name] = row
        shas = {}
        for ver in ("v3", "v4"):
            s = DveOpSpec(name=name, opcode=row, uops=lower(spec, ver=ver),
                          rd1_en=_has_src1(spec))
            shas[ver] = s.sha(ver)
        op = dve_ops.DveOp(name, spec, subdim=False, uops_sha=shas)
        dve_ops.OPS.append(op)
        dve_ops.CUSTOM_DVE_SPECS[name] = spec
        return op

    def _np_not(a):
        return (~np.asarray(a, np.float32).view(np.int32)).view(np.float32)

    from concourse.dve_spec import sq

    # D = (z^2 + C0)*z^2 + C1  — single PSUM read (PSUM has one DVE port)
    yq = sq(Src0)
    deval_spec = Spec(
        body=(yq + C0) * yq + C1,
        reference=lambda in0, s0, s1, imm2: (
            lambda y: (y + s0) * y + s1)(np.square(np.asarray(in0, np.float32))),
    )
    recip_op = mk("KA_DEVAL", deval_spec)

    # r ~= 1/D: bitwise-NOT seed * C0, then two Newton steps. 8 ALU stages.
    seed = Bin(AluOp.BITWISE_NOT, Src0, Src0) * C0
    r1 = seed * ((One + One) - Src0 * seed)
    recip2_spec = Spec(
        body=r1 * ((One + One) - Src0 * r1),
        reference=lambda in0, s0, s1, imm2: (
            lambda Dv: (lambda s: (lambda a: a * (2.0 - Dv * a))(
                s * (2.0 - Dv * s)))(_np_not(Dv) * s0)
        )(np.asarray(in0, np.float32)),
    )
    recip2_op = mk("KA_RECIP2", recip2_spec)

    # g = ((z^2+C0)*z^2) * r * C1    (5 stages; z from PSUM, r elementwise)
    yg = sq(Src0)
    geval_spec = Spec(
        body=((yg + C0) * yg) * Src1 * C1,
        reference=lambda in0, in1, s0, s1, imm2: (
            lambda y: (y + s0) * y * in1 * s1)(
                np.square(np.asarray(in0, np.float32))),
    )
    geval_op = mk("KA_GEVAL", geval_spec)

    # out = (in0 + C0) * in1   (k-stream: (o_k + bg2) * o_q)   2 stages
    kam_spec = Spec(
        body=(Src0 + C0) * Src1,
        reference=lambda in0, in1, s0, s1, imm2: (
            (np.asarray(in0, np.float32) + s0) * in1),
    )
    kam_op = mk("KA_ADDMUL", kam_spec)
    return recip_op, recip2_op, geval_op, kam_op


def _build_nc(reps=1):
    from contextlib import ExitStack
    import concourse.bass as bass
    import concourse.tile as tile
    import concourse.mybir as mybir
    from concourse import bacc

    deval_op, recip2_op, geval_op, kam_op = _register_ops()

    dt = mybir.dt
    f16 = dt.float16
    f32 = dt.float32
    AF = mybir.ActivationFunctionType

    nc = bacc.Bacc(
        "TRN2",
        target_bir_lowering=False,
        debug=False,
        enable_asserts=False,
        num_devices=NCORES,
    )

    ins = {}
    def din(name, shape, dty):
        ins[name] = nc.dram_tensor(name, shape, dty, kind="ExternalInput").ap()
        return ins[name]

    xq = din("xq", [G * (D + 1), BC], f16)      # rows g*65+d (d<64: x^T), row 64: ones
    xk = din("xk", [G * (D + 1), BC], f16)
    # rows g*65+d: [W1[g,d,:] (512) | bypass W1c@W2c/2 (64)], row 64: biases
    w1q = din("w1q", [G * (D + 1), H + P], f16)
    w1k = din("w1k", [G * (D + 1), H + P], f16)
    # pair-packed W2: row j*128+r, col s*256 + hc*64 + p = W2[2j+s, hc*128+r, p]
    w2q = din("w2q", [NPAIR * 128, 512], f16)
    w2k = din("w2k", [NPAIR * 128, 512], f16)
    # rows 0-63: [Wg1 (512) | bypass Wg1c@Wg2c/2 (32)], row 64: bg1 / bypass bias
    wg1 = din("wg1", [D + 1, H + 32], f16)
    wg2 = din("wg2", [128, 4 * 32], f16)        # [r, hc*32+e] = Wg2[hc*128+r, e] (e<16, else 0)
    b2q = din("b2q", [64, G], f32)              # col g = b2[g]
    b2k = din("b2k", [64, G], f32)
    bg2r = din("bg2r", [128, 1], f32)           # 4x [bg2(16); zeros(16)] along partitions
    ones128 = din("ones128", [128, 1], f16)
    onesbc = din("onesbc", [1, BC], f16)

    out = nc.dram_tensor("out", [1, BC], f32, kind="ExternalOutput").ap()

    with tile.TileContext(nc) as tc:
        with ExitStack() as ctx:
            ep = ctx.enter_context
            px = ep(tc.tile_pool(name="px", bufs=8))          # x tiles [65,BC]
            pw1 = ep(tc.tile_pool(name="pw1", bufs=8))        # W1 tiles [65,H+P]
            pw2 = ep(tc.tile_pool(name="pw2", bufs=3))        # W2 pair tiles [128,512]
            phs = ep(tc.tile_pool(name="phs", bufs=8))        # silu'd h [128,512]
            pyr = ep(tc.tile_pool(name="pyr", bufs=4))        # y and r tiles [128,512]
            pfv = ep(tc.tile_pool(name="pfv", bufs=3))        # f pair tiles [128,BC]
            pu = ep(tc.tile_pool(name="pu", bufs=6))          # U tiles [128,BC]
            ph2 = ep(tc.tile_pool(name="ph2", bufs=10))       # silu'd h2 [128,512]
            pbig = ep(tc.tile_pool(name="pbig", bufs=2))      # qs/ks big [128,16*BC]
            pmisc = ep(tc.tile_pool(name="pmisc", bufs=2))
            pconst = ep(tc.tile_pool(name="pconst", bufs=1))
            # PSUM: psh 3 + psc 2 (chunk-3 tiles) + psv 3 = 8 banks
            psh = ep(tc.tile_pool(name="psh", bufs=3, space="PSUM"))
            psc = ep(tc.tile_pool(name="psc", bufs=2, space="PSUM"))
            psv = ep(tc.tile_pool(name="psv", bufs=3, space="PSUM"))
            pdram = ep(tc.tile_pool(name="pdram", bufs=4, space="DRAM"))

            def const_tile(src_ap, shape, dty, name):
                t = pconst.tile(shape, dty, name=name, tag=name)
                nc.sync.dma_start(t[:, :], src_ap)
                return t

            wg1_s = const_tile(wg1, [D + 1, H + 32], f16, "wg1s")
            wg2_s = const_tile(wg2, [128, 4 * 32], f16, "wg2s")
            b2q_s = const_tile(b2q, [64, G], f32, "b2qs")
            b2k_s = const_tile(b2k, [64, G], f32, "b2ks")
            bg2_s = const_tile(bg2r, [128, 1], f32, "bg2s")
            one_s = const_tile(ones128, [128, 1], f16, "ones")
            ones_bc = const_tile(onesbc, [1, BC], f16, "onesbc")

            stream_in = {"q": (xq, w1q, w2q, b2q_s), "k": (xk, w1k, w2k, b2k_s)}

            def dve_silu_chunk(z_sl, fit):
                """3-instr DVE path: g(z) for one [128,512] PSUM slice."""
                n1, c0, d1, d0, cseed = fit
                d_t = pyr.tile([128, 512], f16, tag="y")
                nc.vector._custom_dve(deval_op, out=d_t[:, :], in0=z_sl,
                                      s0=d1, s1=d0)
                r_t = pyr.tile([128, 512], f16, tag="r")
                nc.vector._custom_dve(recip2_op, out=r_t[:, :], in0=d_t[:, :],
                                      s0=cseed)
                g_t = phs.tile([128, 512], f16, tag="hs")
                nc.vector._custom_dve(geval_op, out=g_t[:, :], in0=z_sl,
                                      in1=r_t[:, :], s0=c0, s1=n1)
                return g_t

            # ================= grouped stage =================
            # Software-pipelined: chunk 3 (the DVE-offloadable one) gets its
            # GEMM1 first into the dedicated psc pool and its silu result is
            # consumed one group LATE (GEMM2 c3 + eviction close in the next
            # group's iteration), so no engine ever stalls on the 3-instr DVE
            # chain's latency.
            def grouped(s, fd):
                x_d, w1_d, w2_d, b2_s = stream_in[s]
                carry = [None]   # (v_ps, w2slice_c3, hs3, fv, b2ap, sgi, j)
                fv_done = []     # pair js whose fv got both evictions

                def close_carry():
                    if carry[0] is None:
                        return
                    v_ps_, w2c3, hs3, fv_, b2ap, sgi_, j_ = carry[0]
                    nc.tensor.matmul(v_ps_[:, :], w2c3, hs3[:, :],
                                     start=False, stop=True)
                    nc.vector.tensor_scalar_add(
                        fv_[sgi_ * 64:(sgi_ + 1) * 64, :], v_ps_[:, :], b2ap)
                    if sgi_ == 1:
                        nc.gpsimd.dma_start(
                            fd[j_ * 128:(j_ + 1) * 128, :], fv_[:, :])
                    carry[0] = None

                fv = None
                for g in range(G):
                    j, sgi = divmod(g, 2)
                    off = _offsel(g, OFF_G)
                    if sgi == 0:
                        w2_t = pw2.tile([128, 512], f16, tag="w2")
                        nc.gpsimd.dma_start(
                            w2_t[:, :], w2_d[j * 128:(j + 1) * 128, :])
                        fv = pfv.tile([128, BC], f16, tag="fv")
                    x_t = px.tile([D + 1, BC], f16, tag="x")
                    nc.sync.dma_start(x_t[:, :], x_d[g * 65:(g + 1) * 65, :])
                    w1_t = pw1.tile([D + 1, H + P], f16, tag="w1")
                    nc.sync.dma_start(w1_t[:, :], w1_d[g * 65:(g + 1) * 65, :])
                    v_ps = psv.tile([64, BC], f32, tag="vps")
                    # chunk 3 first, into the dedicated pool
                    hp3 = psc.tile([128, 512], f32, tag="hc3")
                    nc.tensor.matmul(hp3[:, :], w1_t[:, 3 * 128:4 * 128],
                                     x_t[:, :], start=True, stop=True)
                    if off:
                        hs3 = dve_silu_chunk(hp3[:, :], FIT1)
                    else:
                        hs3 = phs.tile([128, 512], f16, tag="hs")
                        nc.scalar.activation(hs3[:, :], hp3[:, :], AF.Silu)
                    # close the PREVIOUS group (its hs3 is ready by now)
                    close_carry()
                    for hc in range(3):
                        hp = psh.tile([128, 512], f32, tag="hps")
                        nc.tensor.matmul(hp[:, :],
                                         w1_t[:, hc * 128:(hc + 1) * 128],
                                         x_t[:, :], start=True, stop=True)
                        hs_t = phs.tile([128, 512], f16, tag="hs")
                        nc.scalar.activation(hs_t[:, :], hp[:, :], AF.Silu)
                        nc.tensor.matmul(
                            v_ps[:, :],
                            w2_t[:, sgi * 256 + hc * 64:sgi * 256 + (hc + 1) * 64],
                            hs_t[:, :],
                            start=(hc == 0), stop=False,
                        )
                    if off:   # z/2 bypass for the offloaded chunk
                        nc.tensor.matmul(v_ps[:, :], w1_t[:, H:H + P],
                                         x_t[:, :], start=False, stop=False)
                    carry[0] = (v_ps,
                                w2_t[:, sgi * 256 + 3 * 64:sgi * 256 + 4 * 64],
                                hs3, fv, b2_s[:, g:g + 1], sgi, j)
                close_carry()

            # ================= global stage =================
            # per patch p: u_p [65, BC] (64 groups + ones row via DMA), bg1
            # rides as wg1_s row 64 so the 512-wide acts are bias-free
            def global_stream(s, fd, big, qbig=None):
                fd3 = fd.rearrange("(g p) b -> p g b", p=P)
                pending = []
                def evict(o_ps_, p__):
                    pr, pcb = 32 * (p__ % 4), (p__ // 4) * BC
                    if qbig is None:
                        nc.vector.tensor_scalar_add(
                            big[pr:pr + 32, pcb:pcb + BC], o_ps_[:, :],
                            bg2_s[pr:pr + 32, 0:1])
                    elif USE_KAM:   # fused (o_k + bg2) * o_q into the k big buffer
                        nc.vector._custom_dve(
                            kam_op,
                            out=big[pr:pr + 32, pcb:pcb + BC],
                            in0=o_ps_[:, :],
                            in1=qbig[pr:pr + 32, pcb:pcb + BC],
                            s0=bg2_s[pr:pr + 32, 0:1])
                    else:
                        nc.vector.tensor_scalar_add(
                            big[pr:pr + 32, pcb:pcb + BC], o_ps_[:, :],
                            bg2_s[pr:pr + 32, 0:1])
                        nc.vector.tensor_mul(
                            big[pr:pr + 32, pcb:pcb + BC],
                            qbig[pr:pr + 32, pcb:pcb + BC],
                            big[pr:pr + 32, pcb:pcb + BC])
                carry = [None]   # (o_ps, hs3, p_)
                def close_carry():
                    if carry[0] is None:
                        return
                    o_ps_, hs3_, p__ = carry[0]
                    nc.tensor.matmul(o_ps_[:, :], wg2_s[:, 3 * 32:4 * 32],
                                     hs3_[:, :], start=False, stop=True)
                    evict(o_ps_, p__)
                    carry[0] = None
                n_off_p = OFF_P if (s == "q" or OFF_P_K is None) else OFF_P_K
                for p_ in range(P):
                    offp = _offsel(p_, n_off_p)
                    u_t = pu.tile([D + 1, BC], f16, tag="u")
                    nc.sync.dma_start(u_t[0:D, :], fd3[p_])
                    nc.vector.tensor_copy(u_t[D:D + 1, :], ones_bc[0:1, :])
                    hp3 = psc.tile([128, 512], f32, tag="hc3")
                    nc.tensor.matmul(hp3[:, :], wg1_s[:, 3 * 128:4 * 128],
                                     u_t[:, :], start=True, stop=True)
                    if offp:
                        hs3 = dve_silu_chunk(hp3[:, :], FIT2)
                    else:
                        hs3 = ph2.tile([128, 512], f16, tag="h2s")
                        nc.scalar.activation(hs3[:, :], hp3[:, :], AF.Silu)
                    close_carry()
                    o_ps = psv.tile([32, BC], f32, tag="vps")
                    for hc in range(3):
                        h2p = psh.tile([128, 512], f32, tag="hps")
                        nc.tensor.matmul(
                            h2p[:, :],
                            wg1_s[:, hc * 128:(hc + 1) * 128],
                            u_t[:, :],
                            start=True, stop=True,
                        )
                        ht = ph2.tile([128, 512], f16, tag="h2s")
                        nc.scalar.activation(ht[:, :], h2p[:, :], AF.Silu)
                        nc.tensor.matmul(
                            o_ps[:, :],
                            wg2_s[:, hc * 32:(hc + 1) * 32],
                            ht[:, :],
                            start=(hc == 0), stop=False,
                        )
                    if offp:   # z/2 bypass for the offloaded chunk
                        nc.tensor.matmul(
                            o_ps[:, :],
                            wg1_s[:, H:H + 32],
                            u_t[:, :],
                            start=False, stop=False,
                        )
                    carry[0] = (o_ps, hs3, p_)
                close_carry()

            for _ in range(reps):
                f_q = pdram.tile([G * P, BC], f16, tag="fq")
                f_k = pdram.tile([G * P, BC], f16, tag="fk")
                grouped("q", f_q)
                grouped("k", f_k)

                qs_big = pbig.tile([128, 16 * BC], f16, tag="qsbig")
                ks_big = pbig.tile([128, 16 * BC], f16, tag="ksbig")
                global_stream("q", f_q, qs_big)
                global_stream("k", f_k, ks_big, qbig=qs_big)

                # ===== logits: reduce q*k product with accumulating matmuls =====
                at_ps = psv.tile([1, BC], f32, tag="vps")
                for c in range(16):
                    nc.tensor.matmul(at_ps[0:1, :], one_s[:, 0:1],
                                     ks_big[:, c * BC:(c + 1) * BC],
                                     start=(c == 0), stop=(c == 15))
                at_s = pmisc.tile([1, BC], f32, tag="at")
                nc.vector.tensor_copy(at_s[0:1, :], at_ps[0:1, :])
                nc.sync.dma_start(out[0:1, :], at_s[0:1, :])

    nc.compile()
    return nc


_NC_CACHE = {}


def _get_nc(reps=1):
    if reps not in _NC_CACHE:
        _NC_CACHE[reps] = _build_nc(reps)
    return _NC_CACHE[reps]


def _prep_inputs(q, k, W1q, b1q, W2q, b2q, W1k, b1k, W2k, b2k, Wg1, bg1, Wg2, bg2):
    f16c = lambda a: np.ascontiguousarray(a, dtype=np.float16)
    f32c = lambda a: np.ascontiguousarray(a, dtype=np.float32)

    def pack_x(x):  # [B, 4096] -> per-core [G*65, BC] with ones row
        shards = []
        for c in range(NCORES):
            xs = np.asarray(x[c * BC:(c + 1) * BC, :], np.float32)
            xt = np.empty((G, D + 1, BC), dtype=np.float16)
            xt[:, :D, :] = xs.T.reshape(G, D, BC).astype(np.float16)
            xt[:, D, :] = 1.0
            shards.append(np.ascontiguousarray(xt.reshape(G * (D + 1), BC)))
        return shards

    def pack_w1(W1, b1, W2):
        W1 = np.asarray(W1, np.float32)
        b1 = np.asarray(b1, np.float32)
        W2 = np.asarray(W2, np.float32)
        w = np.empty((G, D + 1, H + P), dtype=np.float32)
        w[:, :D, :H] = W1
        w[:, D, :H] = b1
        # bypass: z/2 @ W2 for hidden chunk 3 (384:512)
        W1c = W1[:, :, 384:512]            # [G, 64, 128]
        W2c = W2[:, 384:512, :]            # [G, 128, 64]
        w[:, :D, H:] = 0.5 * np.einsum('gdj,gjp->gdp', W1c, W2c)
        w[:, D, H:] = 0.5 * np.einsum('gj,gjp->gp', b1[:, 384:512], W2c)
        return f16c(w.reshape(G * (D + 1), H + P))

    def pack_w2(W2):  # [G, 512, 64] -> [G/2*128, 512] pair-packed
        w = np.asarray(W2, np.float32).reshape(G, 4, 128, 64)   # [g, hc, r, p]
        w = w.transpose(0, 2, 1, 3).reshape(G, 128, 256)        # [g, r, hc*64+p]
        w = w.reshape(G // 2, 2, 128, 256).transpose(0, 2, 1, 3)  # [j, r, s, :]
        return f16c(w.reshape(NPAIR * 128, 512))

    xq_s = pack_x(q)
    xk_s = pack_x(k)
    w1q_p = pack_w1(W1q, b1q, W2q)
    w1k_p = pack_w1(W1k, b1k, W2k)
    w2q_p = pack_w2(W2q)
    w2k_p = pack_w2(W2k)
    b2q_p = f32c(np.asarray(b2q, np.float32).T)                 # [64(P), G]
    b2k_p = f32c(np.asarray(b2k, np.float32).T)

    Wg1f = np.asarray(Wg1, np.float32)
    bg1f = np.asarray(bg1, np.float32)
    Wg2f = np.asarray(Wg2, np.float32)
    wg1_p = np.zeros((D + 1, H + 32), dtype=np.float32)
    wg1_p[:D, :H] = Wg1f
    wg1_p[D, :H] = bg1f
    wg1_p[:D, H:H + E] = 0.5 * (Wg1f[:, 384:512] @ Wg2f[384:512, :])
    wg1_p[D, H:H + E] = 0.5 * (bg1f[384:512] @ Wg2f[384:512, :])
    wg1_p = f16c(wg1_p)

    wg2_p = np.zeros((128, 4, 32), dtype=np.float16)
    wg2_p[:, :, :E] = Wg2f.reshape(4, 128, E).transpose(1, 0, 2).astype(np.float16)
    wg2_p = np.ascontiguousarray(wg2_p.reshape(128, 4 * 32))    # [r, hc*32+e]
    bg2_p = np.zeros((4, 32), dtype=np.float32)
    bg2_p[:, :E] = np.asarray(bg2, np.float32)
    bg2_p = f32c(bg2_p.reshape(128, 1))
    ones_p = np.ones((128, 1), dtype=np.float16)

    in_maps = []
    for c in range(NCORES):
        in_maps.append({
            "xq": xq_s[c], "xk": xk_s[c],
            "w1q": w1q_p, "w1k": w1k_p,
            "w2q": w2q_p, "w2k": w2k_p,
            "wg1": wg1_p, "wg2": wg2_p,
            "b2q": b2q_p, "b2k": b2k_p,
            "bg2r": bg2_p, "ones128": ones_p,
            "onesbc": np.ones((1, BC), dtype=np.float16),
        })
    return in_maps


def kernel(q, k, W1q, b1q, W2q, b2q, W1k, b1k, W2k, b2k, Wg1, bg1, Wg2, bg2,
           _trace=False, _tracedir=None):
    from concourse.bass_utils import run_bass_kernel_spmd

    in_maps = _prep_inputs(q, k, W1q, b1q, W2q, b2q, W1k, b1k, W2k, b2k,
                           Wg1, bg1, Wg2, bg2)
    nc = _get_nc()
    kw = {}
    if _trace:
        kw = dict(trace=True, tmpdir=_tracedir)
    res = run_bass_kernel_spmd(nc, in_maps, core_ids=list(range(NCORES)), **kw)
    logits = np.concatenate([res.results[c]["out"].reshape(BC)
                             for c in range(NCORES)]).astype(np.float64)
    m = logits.max()
    e = np.exp(logits - m)
    sm = (e / e.sum()).astype(np.float32)
    if _trace:
        kernel._last_trace = res
    return sm
